# revision 25
# baseline (speedup 1.0000x reference)
"""Trainium2 Bass kernel for multi-level bilinear grid interpolation
(embedding_lookup, nn_COOLCHIC_INTERP_ENC).

Strategy (v3):
  - 8 NeuronCores, data-parallel over query points, sharded spatially by
    latitude into 256 count-balanced bands (8 ranks x 4 passes x 8 gpsimd
    cores; equal-angle fallback for pathological inputs).
  - KEY TRICK: floor(t/res_l) == floor(t/res_0) >> l exactly (res_l are
    powers of two and t/res_l is an exact f32 scaling), so ONE level-0
    cell index (row-in-band, col) identifies every level's bilinear quad.
    ap_gather reads each partition's own table row, so partition 16k+q of
    gpsimd core k holds a table for level q%8 whose entry e is that
    level's 2x2 quad for L0-cell e -> a single d=1 f32 ap_gather per
    batch fetches ALL 8 levels' quads for the core's 16*F points.
  - Quads are 4 x int8 (per-level symmetric quantization, error
    <= absmax/254 ~ 0.4%) packed in one f32 word.
  - Gather indices (int16) and per-level lerp fractions (fp16) are
    host-precomputed directly in engine layouts; the per-level gather
    output de-interleave is ONE SBUF->SBUF DMA with 2KB-contiguous
    descriptors (partition 16k+l holds level l's value for the whole
    core stream; stream slot q*F+j belongs to lerp partition 16k+q).
  - DVE does the 9-op bilinear lerp per level (int8 corners, f32
    intermediates, fp16 fracs/result); host de-quantizes.
"""

import sys

sys.path.insert(0, "/opt/trn_rl_repo")

import numpy as np

from concourse import bacc, bass, mybir
import concourse.tile as tile

# ---------------------------------------------------------------- constants
H_GRID, W_GRID, LEVEL, RES = 721, 1440, 8, 0.25
N_RANKS = 8
N_PASSES = 4
N_Q7 = 8
BANDS = N_RANKS * N_PASSES * N_Q7  # 256
F = 496                   # points per partition per batch
NI = 16 * F               # gather stream length per core (= points/core/batch)
CAP0 = 4                  # level-0 rows per band (max floor-span, exact)
ETOT = CAP0 * W_GRID      # table entries per partition (L0 cells)
NMC = 1 + 2 * LEVEL       # meta channels: idx + (fa, fb) per level

F32 = mybir.dt.float32
F16 = mybir.dt.float16
I16 = mybir.dt.int16
I8 = mybir.dt.int8


def _res(l):
    return RES * (2.0 ** l)


# ---------------------------------------------------------------- device kernel
def build_kernel(n_batch):
    """Per-rank SPMD Bass program. c_band = n_batch * NI points per band."""
    nc = bacc.Bacc(None, target_bir_lowering=False)

    # tables come 16x-per-level-replicated from the host: [8(k), 16(q), ETOT]
    # with partition 16k+q holding band k's level-(q%8) quad table.
    tab_t = nc.declare_dram_parameter(
        "tab", [N_PASSES, N_Q7, 16, ETOT], F32, False)
    idx_t = nc.declare_dram_parameter(
        "idx", [N_PASSES, n_batch, 128, F], I16, False)
    frc_t = nc.declare_dram_parameter(
        "frc", [N_PASSES, n_batch, 128, 2 * LEVEL, F], F16, False)
    out_t = nc.declare_dram_parameter(
        "out", [N_PASSES, n_batch, 128, LEVEL, F], F16, True)

    sub = mybir.AluOpType.subtract
    add = mybir.AluOpType.add
    mult = mybir.AluOpType.mult

    from contextlib import ExitStack

    nbat = N_PASSES * n_batch           # global batch index g = p*n_batch+bi
    LOOK = 2                            # deint/unpack emitted LOOK items early

    with tile.TileContext(nc) as tc, ExitStack() as es:
        ptab = es.enter_context(tc.tile_pool(name="ptab", bufs=2))
        pdst = es.enter_context(tc.tile_pool(name="pdst", bufs=2))
        pm = es.enter_context(tc.tile_pool(name="pm", bufs=2))
        pq = es.enter_context(tc.tile_pool(name="pq", bufs=3))
        pr = es.enter_context(tc.tile_pool(name="pr", bufs=2))
        pt = es.enter_context(tc.tile_pool(name="pt", bufs=3))

        # per-global-batch state created lazily in emission order
        tabs_of, dst_of, fr_of, res_of, crn_of = {}, {}, {}, {}, {}

        def emit_batch_front(g):
            """prefetch idx/frc, (new pass: table), and the gather for g."""
            p, bi = divmod(g, n_batch)
            if bi == 0:
                tabs = ptab.tile([128, ETOT], F32, tag="tabs")
                for c in range(4):
                    nc.sync.dma_start(out=tabs[32 * c:32 * (c + 1)],
                                      in_=tab_t[p, 2 * c:2 * c + 2])
                tabs_of[p] = tabs
            ix = pm.tile([128, F], I16, tag="idx")
            nc.sync.dma_start(out=ix[:], in_=idx_t[p, bi])
            fr = pm.tile([128, 2 * LEVEL, F], F16, tag="frc")
            nc.sync.dma_start(out=fr[:], in_=frc_t[p, bi])
            fr_of[g] = fr
            dst = pdst.tile([128, NI], F32, tag="dst")
            nc.gpsimd.ap_gather(
                dst[:].rearrange("p (n d) -> p n d", d=1),
                tabs_of[p][:].rearrange("p (n d) -> p n d", d=1),
                ix[:],
                channels=128, num_elems=ETOT, d=1, num_idxs=NI)
            dst_of[g] = dst

        def emit_fetch(g, l):
            """de-interleave levels l, l+1 of batch g + int8->fp16 corner
            unpack, all on Activation so their gather-wait can't block SP."""
            quad = pq.tile([128, 2, F], F32, tag="quad")
            nc.scalar.dma_start(out=quad[:, 0, :], in_=dst_of[g][l::16])
            nc.scalar.dma_start(out=quad[:, 1, :], in_=dst_of[g][l + 1::16])
            crn = pq.tile([128, 4, 2 * F], F16, tag="crn")
            nc.scalar.copy(out=crn[:].rearrange("p r j -> p j r"),
                           in_=quad[:].rearrange("p v j -> p (v j)").bitcast(I8))
            crn_of[(g, l)] = crn

        def emit_lerp(g, l):
            p, bi = divmod(g, n_batch)
            crn = crn_of.pop((g, l))
            v00, v10, v01, v11 = (crn[:, c, :] for c in range(4))
            fr = fr_of[g]
            fa = fr[:, l:l + 2, :].rearrange("p v j -> p (v j)")
            fb = fr[:, LEVEL + l:LEVEL + l + 2, :].rearrange(
                "p v j -> p (v j)")
            t1 = pt.tile([128, 2 * F], F16, tag="t1")
            t2 = pt.tile([128, 2 * F], F16, tag="t2")
            res = pr.tile([128, 2, F], F16, tag="res")
            # offload ~1/10 of the lerps to the (mostly idle) gpsimd engine
            # to shave the DVE critical path.
            V = nc.gpsimd if (g % 3 == 1 and l == 6) else nc.vector
            # v_f = v00 + fb*(v01 - v00); v_c = v10 + fb*(v11 - v10)
            V.tensor_tensor(out=t1[:], in0=v01, in1=v00, op=sub)
            V.tensor_tensor(out=t1[:], in0=t1[:], in1=fb, op=mult)
            V.tensor_tensor(out=t1[:], in0=t1[:], in1=v00, op=add)
            V.tensor_tensor(out=t2[:], in0=v11, in1=v10, op=sub)
            V.tensor_tensor(out=t2[:], in0=t2[:], in1=fb, op=mult)
            V.tensor_tensor(out=t2[:], in0=t2[:], in1=v10, op=add)
            # out = v_f + fa*(v_c - v_f)
            V.tensor_tensor(out=t2[:], in0=t2[:], in1=t1[:], op=sub)
            V.tensor_tensor(out=t2[:], in0=t2[:], in1=fa, op=mult)
            V.tensor_tensor(out=res[:].rearrange("p v j -> p (v j)"),
                            in0=t2[:], in1=t1[:], op=add)
            nc.sync.dma_start(out=out_t[p, bi, :, l:l + 2, :], in_=res[:])
            if l == LEVEL - 2:
                fr_of.pop(g)

        items = [(g, l) for g in range(nbat) for l in range(0, LEVEL, 2)]
        emit_batch_front(0)
        for k in range(len(items) + LOOK):
            if k < len(items):
                g, l = items[k]
                # keep the NEXT batch's gather a full batch ahead
                if l == 0 and g + 1 < nbat:
                    emit_batch_front(g + 1)
                emit_fetch(g, l)
            if k >= LOOK:
                emit_lerp(*items[k - LOOK])

    nc.compile()
    return nc


# ---------------------------------------------------------------- host tables
def quantize(emb):
    """emb [LEVEL,H,W] f32 -> int8 grids + per-level dequant factors."""
    scl = np.abs(emb).max(axis=(1, 2))
    scl = np.where(scl > 0, scl, 1.0).astype(np.float64)
    q8 = np.clip(np.rint(emb * (127.0 / scl)[:, None, None]),
                 -127, 127).astype(np.int8)
    return q8, (scl / 127.0).astype(np.float64)


def equal_angle_boundaries():
    """Band boundaries in t = 90 - lat space; exact f32 values."""
    return np.float32(np.arange(1, BANDS) * 45.0 / 64.0)


def quantile_boundaries(t32):
    """Count-balanced boundaries: the sorted t32 at the 256-quantile cuts."""
    ts = np.sort(t32)
    return ts[(np.arange(1, BANDS, dtype=np.int64) * ts.size) // BANDS]


def band_row_starts(bnd):
    """RS0[b] = floor(lo_b / RES), exact: lo_b is an exact f32 and 1/RES a
    power of two, so the f64 product is exact."""
    lo = np.concatenate([[np.float32(0.0)], bnd]).astype(np.float64)
    return np.floor(lo / RES).astype(np.int64)


def build_tables(q8, RS0):
    """-> tab [BANDS, 16, ETOT] f32; partition q holds level q%8's quad
    table over L0 cells: entry (r0loc, w0) = level-l quad at
    (h_l, w_l) = ((RS0+r0loc)>>l, w0>>l), int8x4-packed."""
    tab = np.zeros((BANDS, N_Q7, ETOT, 4), np.int8)  # [band, level, e, 4]
    w0 = np.arange(W_GRID)
    for l in range(LEVEL):
        rows0 = RS0[:, None] + np.arange(CAP0)[None, :]       # [BANDS, CAP0]
        hl = np.clip(rows0 >> l, 0, H_GRID - 1)
        hl1 = np.clip((rows0 >> l) + 1, 0, H_GRID - 1)
        wl = w0 >> l
        wl1 = np.minimum(wl + 1, W_GRID - 1)
        g0 = q8[l][hl]                                        # [BANDS,CAP0,W]
        g1 = q8[l][hl1]
        ent = np.stack([g0[:, :, wl], g1[:, :, wl], g0[:, :, wl1],
                        g1[:, :, wl1]], axis=-1)              # [B,CAP0,W,4]
        tab[:, l] = ent.reshape(BANDS, ETOT, 4)
    # replicate levels onto partitions 8..15, view as f32 words
    tab16 = np.concatenate([tab, tab], axis=1)                # [B, 16, E, 4]
    return np.ascontiguousarray(tab16).view('<f4').reshape(BANDS, 16, ETOT)


# ---------------------------------------------------------------- host points
def point_data(t32, lon, band, RS0):
    """idx int16 [N] (L0 cell id in band window) + per-level fracs fp16,
    plus in-window validity. Mirrors the f32 reference exactly."""
    a0 = t32 / np.float32(RES)
    fl0 = np.floor(a0)
    raw = np.clip(fl0, 0, H_GRID - 1).astype(np.int64) - RS0[band]
    ok = bool(raw.size == 0 or (raw.min() >= 0 and raw.max() <= CAP0 - 1))
    row_local = np.clip(raw, 0, CAP0 - 1)
    o0 = lon / np.float32(RES)
    w0 = np.clip(np.floor(o0), 0, W_GRID - 1).astype(np.int64)
    idx = (row_local * W_GRID + w0).astype(np.int16)
    fas, fbs = [], []
    for l in range(LEVEL):
        r = np.float32(_res(l))
        a = t32 / r
        lat_f = np.clip(np.floor(a), 0, H_GRID - 1)
        fas.append((a - lat_f).astype(np.float16))
        o = lon / r
        wf = np.clip(np.floor(o), 0, W_GRID - 1)
        fbs.append((o - wf).astype(np.float16))
    return idx, fas, fbs, ok


def slot_assign(band, c_band):
    """slot_global [N]: slot index in [0, BANDS*c_band) per point."""
    order = np.argsort(band, kind="stable")
    counts = np.bincount(band, minlength=BANDS)
    starts = np.zeros(BANDS, np.int64)
    starts[1:] = np.cumsum(counts)[:-1]
    pos_sorted = np.arange(band.size, dtype=np.int64) - starts[band[order]]
    slot_global = np.empty(band.size, np.int64)
    slot_global[order] = band[order] * c_band + pos_sorted
    return slot_global, counts


def _to_lerp_layout(slots, n_batch):
    """[BANDS, c_band] -> [BANDS, nb, 16(q), F(j)]; slot s=(bi*F+j)*16+q."""
    return (slots.reshape(BANDS, n_batch, F, 16).transpose(0, 1, 3, 2))


def _to_idx_layout(slots, n_batch):
    """[BANDS, c_band] -> [BANDS, nb, 16(m), F(c)]; stream i = q*F+j,
    written at partition m=i%16, col c=i//16."""
    lerp = _to_lerp_layout(slots, n_batch)          # [B, nb, q, j]
    stream = lerp.reshape(BANDS, n_batch, NI)       # i = q*F + j
    return stream.reshape(BANDS, n_batch, F, 16).transpose(0, 1, 3, 2)


# ---------------------------------------------------------------- entry point
_NC_CACHE = {}
LAST_RESULT = None


def kernel(x, embeddings):
    global LAST_RESULT
    from concourse.bass_utils import run_bass_kernel_spmd

    x = np.ascontiguousarray(np.asarray(x), dtype=np.float32)
    emb = np.asarray(embeddings, dtype=np.float32)
    n = x.shape[0]

    q8, deq = quantize(emb)
    lat = x[:, 0].astype(np.float32)
    lon = x[:, 1].astype(np.float32)
    t32 = np.float32(90.0) - lat

    # count-balanced bands minimize padded-slot waste; fall back to
    # equal-angle bands if any L0 floor escapes its band's 4-row window
    # (only possible for pathological latitude distributions).
    for bnd in (quantile_boundaries(t32), equal_angle_boundaries()):
        band = np.searchsorted(bnd, t32, side="right").astype(np.int64)
        RS0 = band_row_starts(bnd)
        idx, fas, fbs, ok = point_data(t32, lon, band, RS0)
        if ok:
            break
    tab = build_tables(q8, RS0)                     # [BANDS, 16, ETOT] f32

    counts = np.bincount(band, minlength=BANDS)
    n_batch = 1
    while n_batch * NI < counts.max():
        n_batch += 1
    c_band = n_batch * NI

    if n_batch not in _NC_CACHE:
        _NC_CACHE[n_batch] = build_kernel(n_batch)
    nc = _NC_CACHE[n_batch]

    slot_global, counts = slot_assign(band, c_band)

    idxm = np.zeros((BANDS, n_batch, 16, F), np.int16)
    sl = np.zeros(BANDS * c_band, np.int16)
    sl[slot_global] = idx
    idxm[:] = _to_idx_layout(sl.reshape(BANDS, c_band), n_batch)
    frcm = np.zeros((BANDS, n_batch, 16, 2 * LEVEL, F), np.float16)
    for l in range(LEVEL):
        for ch, v in ((l, fas[l]), (LEVEL + l, fbs[l])):
            sf = np.zeros(BANDS * c_band, np.float16)
            sf[slot_global] = v
            frcm[:, :, :, ch, :] = _to_lerp_layout(
                sf.reshape(BANDS, c_band), n_batch)

    # bands -> (rank, pass, core): band = 32r + 8p + k
    tab_r = tab.reshape(N_RANKS, N_PASSES, N_Q7, 16, ETOT)
    idx_r = (idxm.reshape(N_RANKS, N_PASSES, N_Q7, n_batch, 16, F)
             .transpose(0, 1, 3, 2, 4, 5)
             .reshape(N_RANKS, N_PASSES, n_batch, 128, F))
    frc_r = (frcm.reshape(N_RANKS, N_PASSES, N_Q7, n_batch, 16, 2 * LEVEL, F)
             .transpose(0, 1, 3, 2, 4, 5, 6)
             .reshape(N_RANKS, N_PASSES, n_batch, 128, 2 * LEVEL, F))

    in_maps = [
        {"tab": np.ascontiguousarray(tab_r[r]),
         "idx": np.ascontiguousarray(idx_r[r]),
         "frc": np.ascontiguousarray(frc_r[r])}
        for r in range(N_RANKS)
    ]
    kres = run_bass_kernel_spmd(nc, in_maps, list(range(N_RANKS)))
    LAST_RESULT = kres
    results = kres.results
    res = np.stack([results[r]["out"] for r in range(N_RANKS)])
    # [R, P, nb, 128(k,q), L, F] -> [BANDS, c_band(bi,j,q), LEVEL]
    res = (res.reshape(N_RANKS, N_PASSES, n_batch, N_Q7, 16, LEVEL, F)
           .transpose(0, 1, 3, 2, 6, 4, 5)
           .reshape(BANDS * c_band, LEVEL))

    out = res[slot_global].astype(np.float32) * \
        (np.asarray(deq, np.float32)[None, :])
    assert out.shape == (n, LEVEL)
    return out


# revision 28
# speedup vs baseline: 1.1290x; 1.1290x over previous
"""Trainium2 Bass kernel for multi-level bilinear grid interpolation
(embedding_lookup, nn_COOLCHIC_INTERP_ENC).

Strategy (v3):
  - 8 NeuronCores, data-parallel over query points, sharded spatially by
    latitude into 256 count-balanced bands (8 ranks x 4 passes x 8 gpsimd
    cores; equal-angle fallback for pathological inputs).
  - KEY TRICK: floor(t/res_l) == floor(t/res_0) >> l exactly (res_l are
    powers of two and t/res_l is an exact f32 scaling), so ONE level-0
    cell index (row-in-band, col) identifies every level's bilinear quad.
    ap_gather reads each partition's own table row, so partition 16k+q of
    gpsimd core k holds a table for level q%8 whose entry e is that
    level's 2x2 quad for L0-cell e -> a single d=1 f32 ap_gather per
    batch fetches ALL 8 levels' quads for the core's 16*F points.
  - Quads are 4 x int8 (per-level symmetric quantization, error
    <= absmax/254 ~ 0.4%) packed in one f32 word.
  - Gather indices (int16) and per-level lerp fractions (fp16) are
    host-precomputed directly in engine layouts; the per-level gather
    output de-interleave is ONE SBUF->SBUF DMA with 2KB-contiguous
    descriptors (partition 16k+l holds level l's value for the whole
    core stream; stream slot q*F+j belongs to lerp partition 16k+q).
  - DVE does the 9-op bilinear lerp per level (int8 corners, f32
    intermediates, fp16 fracs/result); host de-quantizes.
"""

import sys

sys.path.insert(0, "/opt/trn_rl_repo")

import numpy as np

from concourse import bacc, bass, mybir
import concourse.tile as tile

# ---------------------------------------------------------------- constants
H_GRID, W_GRID, LEVEL, RES = 721, 1440, 8, 0.25
N_RANKS = 8
N_PASSES = 4
N_Q7 = 8
BANDS = N_RANKS * N_PASSES * N_Q7  # 256
F = 496                   # points per partition per batch
NI = 16 * F               # gather stream length per core (= points/core/batch)
CAP0 = 4                  # level-0 rows per band (max floor-span, exact)
ETOT = CAP0 * W_GRID      # table entries per partition (L0 cells)
NMC = 1 + 2 * LEVEL       # meta channels: idx + (fa, fb) per level

F32 = mybir.dt.float32
F16 = mybir.dt.float16
I16 = mybir.dt.int16
I8 = mybir.dt.int8


def _res(l):
    return RES * (2.0 ** l)


# ---------------------------------------------------------------- device kernel
def build_kernel(n_batch):
    """Per-rank SPMD Bass program. c_band = n_batch * NI points per band."""
    nc = bacc.Bacc(None, target_bir_lowering=False)

    # tables come 16x-per-level-replicated from the host: [8(k), 16(q), ETOT]
    # with partition 16k+q holding band k's level-(q%8) quad table.
    tab_t = nc.declare_dram_parameter(
        "tab", [N_PASSES, N_Q7, 16, ETOT], F32, False)
    idx_t = nc.declare_dram_parameter(
        "idx", [N_PASSES, n_batch, 128, F], I16, False)
    frc_t = nc.declare_dram_parameter(
        "frc", [N_PASSES, n_batch, 128, 2 * LEVEL, F], F16, False)
    out_t = nc.declare_dram_parameter(
        "out", [N_PASSES, n_batch, 128, LEVEL, F], F16, True)

    sub = mybir.AluOpType.subtract
    add = mybir.AluOpType.add
    mult = mybir.AluOpType.mult

    from contextlib import ExitStack

    nbat = N_PASSES * n_batch           # global batch index g = p*n_batch+bi
    LOOK = 2                            # deint/unpack emitted LOOK items early

    with tile.TileContext(nc) as tc, ExitStack() as es:
        ptab = es.enter_context(tc.tile_pool(name="ptab", bufs=2))
        pdst = es.enter_context(tc.tile_pool(name="pdst", bufs=2))
        pm = es.enter_context(tc.tile_pool(name="pm", bufs=2))
        pq = es.enter_context(tc.tile_pool(name="pq", bufs=3))
        pr = es.enter_context(tc.tile_pool(name="pr", bufs=2))
        pt = es.enter_context(tc.tile_pool(name="pt", bufs=3))

        # per-global-batch state created lazily in emission order
        tabs_of, dst_of, fr_of, res_of, crn_of = {}, {}, {}, {}, {}

        def emit_batch_front(g):
            """prefetch idx/frc, (new pass: table), and the gather for g."""
            p, bi = divmod(g, n_batch)
            ix = pm.tile([128, F], I16, tag="idx")
            nc.sync.dma_start(out=ix[:], in_=idx_t[p, bi])
            if bi == 0:
                # NOTE: all 16 partition rows must hold valid tables — the
                # gather ucode mixes the two replicated level copies (q and
                # q+8), so garbage there corrupts consumed outputs.
                tabs = ptab.tile([128, ETOT], F32, tag="tabs")
                for c in range(4):
                    nc.sync.dma_start(out=tabs[32 * c:32 * (c + 1)],
                                      in_=tab_t[p, 2 * c:2 * c + 2])
                tabs_of[p] = tabs
            fr = pm.tile([128, 2 * LEVEL, F], F16, tag="frc")
            nc.sync.dma_start(out=fr[:], in_=frc_t[p, bi])
            fr_of[g] = fr
            dst = pdst.tile([128, NI], F32, tag="dst")
            nc.gpsimd.ap_gather(
                dst[:].rearrange("p (n d) -> p n d", d=1),
                tabs_of[p][:].rearrange("p (n d) -> p n d", d=1),
                ix[:],
                channels=128, num_elems=ETOT, d=1, num_idxs=NI)
            dst_of[g] = dst

        def emit_fetch(g, l):
            """de-interleave levels l, l+1 of batch g + int8->fp16 corner
            unpack, all on Activation so their gather-wait can't block SP."""
            quad = pq.tile([128, 2, F], F32, tag="quad")
            nc.scalar.dma_start(out=quad[:, 0, :], in_=dst_of[g][l::16])
            nc.scalar.dma_start(out=quad[:, 1, :], in_=dst_of[g][l + 1::16])
            crn = pq.tile([128, 4, 2 * F], F16, tag="crn")
            nc.scalar.copy(out=crn[:].rearrange("p r j -> p j r"),
                           in_=quad[:].rearrange("p v j -> p (v j)").bitcast(I8))
            crn_of[(g, l)] = crn

        def emit_lerp(g, l):
            p, bi = divmod(g, n_batch)
            crn = crn_of.pop((g, l))
            v00, v10, v01, v11 = (crn[:, c, :] for c in range(4))
            fr = fr_of[g]
            fa = fr[:, l:l + 2, :].rearrange("p v j -> p (v j)")
            fb = fr[:, LEVEL + l:LEVEL + l + 2, :].rearrange(
                "p v j -> p (v j)")
            t1 = pt.tile([128, 2 * F], F16, tag="t1")
            t2 = pt.tile([128, 2 * F], F16, tag="t2")
            res = pr.tile([128, 2, F], F16, tag="res")
            V = nc.vector
            # v_f = v00 + fb*(v01 - v00); v_c = v10 + fb*(v11 - v10)
            V.tensor_tensor(out=t1[:], in0=v01, in1=v00, op=sub)
            V.tensor_tensor(out=t1[:], in0=t1[:], in1=fb, op=mult)
            V.tensor_tensor(out=t1[:], in0=t1[:], in1=v00, op=add)
            V.tensor_tensor(out=t2[:], in0=v11, in1=v10, op=sub)
            V.tensor_tensor(out=t2[:], in0=t2[:], in1=fb, op=mult)
            V.tensor_tensor(out=t2[:], in0=t2[:], in1=v10, op=add)
            # out = v_f + fa*(v_c - v_f)
            V.tensor_tensor(out=t2[:], in0=t2[:], in1=t1[:], op=sub)
            V.tensor_tensor(out=t2[:], in0=t2[:], in1=fa, op=mult)
            V.tensor_tensor(out=res[:].rearrange("p v j -> p (v j)"),
                            in0=t2[:], in1=t1[:], op=add)
            nc.sync.dma_start(out=out_t[p, bi, :, l:l + 2, :], in_=res[:])
            if l == LEVEL - 2:
                fr_of.pop(g)

        items = [(g, l) for g in range(nbat) for l in range(0, LEVEL, 2)]
        emit_batch_front(0)
        for k in range(len(items) + LOOK):
            if k < len(items):
                g, l = items[k]
                # keep the NEXT batch's gather a full batch ahead
                if l == 0 and g + 1 < nbat:
                    emit_batch_front(g + 1)
                emit_fetch(g, l)
            if k >= LOOK:
                emit_lerp(*items[k - LOOK])

    nc.compile()
    return nc


# ---------------------------------------------------------------- host tables
def quantize(emb):
    """emb [LEVEL,H,W] f32 -> int8 grids + per-level dequant factors."""
    scl = np.abs(emb).max(axis=(1, 2))
    scl = np.where(scl > 0, scl, 1.0).astype(np.float64)
    q8 = np.clip(np.rint(emb * (127.0 / scl)[:, None, None]),
                 -127, 127).astype(np.int8)
    return q8, (scl / 127.0).astype(np.float64)


def equal_angle_boundaries():
    """Band boundaries in t = 90 - lat space; exact f32 values."""
    return np.float32(np.arange(1, BANDS) * 45.0 / 64.0)


def quantile_boundaries(t32):
    """Count-balanced boundaries: the sorted t32 at the 256-quantile cuts."""
    ts = np.sort(t32)
    return ts[(np.arange(1, BANDS, dtype=np.int64) * ts.size) // BANDS]


def band_row_starts(bnd):
    """RS0[b] = floor(lo_b / RES), exact: lo_b is an exact f32 and 1/RES a
    power of two, so the f64 product is exact."""
    lo = np.concatenate([[np.float32(0.0)], bnd]).astype(np.float64)
    return np.floor(lo / RES).astype(np.int64)


def build_tables(q8, RS0):
    """-> tab [BANDS, 16, ETOT] f32; partition q holds level q%8's quad
    table over L0 cells: entry (r0loc, w0) = level-l quad at
    (h_l, w_l) = ((RS0+r0loc)>>l, w0>>l), int8x4-packed."""
    tab = np.zeros((BANDS, N_Q7, ETOT, 4), np.int8)  # [band, level, e, 4]
    w0 = np.arange(W_GRID)
    for l in range(LEVEL):
        rows0 = RS0[:, None] + np.arange(CAP0)[None, :]       # [BANDS, CAP0]
        hl = np.clip(rows0 >> l, 0, H_GRID - 1)
        hl1 = np.clip((rows0 >> l) + 1, 0, H_GRID - 1)
        wl = w0 >> l
        wl1 = np.minimum(wl + 1, W_GRID - 1)
        g0 = q8[l][hl]                                        # [BANDS,CAP0,W]
        g1 = q8[l][hl1]
        ent = np.stack([g0[:, :, wl], g1[:, :, wl], g0[:, :, wl1],
                        g1[:, :, wl1]], axis=-1)              # [B,CAP0,W,4]
        tab[:, l] = ent.reshape(BANDS, ETOT, 4)
    # replicate levels onto partitions 8..15, view as f32 words
    tab16 = np.concatenate([tab, tab], axis=1)                # [B, 16, E, 4]
    return np.ascontiguousarray(tab16).view('<f4').reshape(BANDS, 16, ETOT)


# ---------------------------------------------------------------- host points
def point_data(t32, lon, band, RS0):
    """idx int16 [N] (L0 cell id in band window) + per-level fracs fp16,
    plus in-window validity. Mirrors the f32 reference exactly."""
    a0 = t32 / np.float32(RES)
    fl0 = np.floor(a0)
    raw = np.clip(fl0, 0, H_GRID - 1).astype(np.int64) - RS0[band]
    ok = bool(raw.size == 0 or (raw.min() >= 0 and raw.max() <= CAP0 - 1))
    row_local = np.clip(raw, 0, CAP0 - 1)
    o0 = lon / np.float32(RES)
    w0 = np.clip(np.floor(o0), 0, W_GRID - 1).astype(np.int64)
    idx = (row_local * W_GRID + w0).astype(np.int16)
    fas, fbs = [], []
    for l in range(LEVEL):
        r = np.float32(_res(l))
        a = t32 / r
        lat_f = np.clip(np.floor(a), 0, H_GRID - 1)
        fas.append((a - lat_f).astype(np.float16))
        o = lon / r
        wf = np.clip(np.floor(o), 0, W_GRID - 1)
        fbs.append((o - wf).astype(np.float16))
    return idx, fas, fbs, ok


def slot_assign(band, c_band):
    """slot_global [N]: slot index in [0, BANDS*c_band) per point."""
    order = np.argsort(band, kind="stable")
    counts = np.bincount(band, minlength=BANDS)
    starts = np.zeros(BANDS, np.int64)
    starts[1:] = np.cumsum(counts)[:-1]
    pos_sorted = np.arange(band.size, dtype=np.int64) - starts[band[order]]
    slot_global = np.empty(band.size, np.int64)
    slot_global[order] = band[order] * c_band + pos_sorted
    return slot_global, counts


def _to_lerp_layout(slots, n_batch):
    """[BANDS, c_band] -> [BANDS, nb, 16(q), F(j)]; slot s=(bi*F+j)*16+q."""
    return (slots.reshape(BANDS, n_batch, F, 16).transpose(0, 1, 3, 2))


def _to_idx_layout(slots, n_batch):
    """[BANDS, c_band] -> [BANDS, nb, 16(m), F(c)]; stream i = q*F+j,
    written at partition m=i%16, col c=i//16."""
    lerp = _to_lerp_layout(slots, n_batch)          # [B, nb, q, j]
    stream = lerp.reshape(BANDS, n_batch, NI)       # i = q*F + j
    return stream.reshape(BANDS, n_batch, F, 16).transpose(0, 1, 3, 2)


# ---------------------------------------------------------------- entry point
_NC_CACHE = {}
LAST_RESULT = None


def kernel(x, embeddings):
    global LAST_RESULT
    from concourse.bass_utils import run_bass_kernel_spmd

    x = np.ascontiguousarray(np.asarray(x), dtype=np.float32)
    emb = np.asarray(embeddings, dtype=np.float32)
    n = x.shape[0]

    q8, deq = quantize(emb)
    lat = x[:, 0].astype(np.float32)
    lon = x[:, 1].astype(np.float32)
    t32 = np.float32(90.0) - lat

    # count-balanced bands minimize padded-slot waste; fall back to
    # equal-angle bands if any L0 floor escapes its band's 4-row window
    # (only possible for pathological latitude distributions).
    for bnd in (quantile_boundaries(t32), equal_angle_boundaries()):
        band = np.searchsorted(bnd, t32, side="right").astype(np.int64)
        RS0 = band_row_starts(bnd)
        idx, fas, fbs, ok = point_data(t32, lon, band, RS0)
        if ok:
            break
    tab = build_tables(q8, RS0)                     # [BANDS, 16, ETOT] f32

    counts = np.bincount(band, minlength=BANDS)
    n_batch = 1
    while n_batch * NI < counts.max():
        n_batch += 1
    c_band = n_batch * NI

    if n_batch not in _NC_CACHE:
        _NC_CACHE[n_batch] = build_kernel(n_batch)
    nc = _NC_CACHE[n_batch]

    slot_global, counts = slot_assign(band, c_band)

    idxm = np.zeros((BANDS, n_batch, 16, F), np.int16)
    sl = np.zeros(BANDS * c_band, np.int16)
    sl[slot_global] = idx
    idxm[:] = _to_idx_layout(sl.reshape(BANDS, c_band), n_batch)
    frcm = np.zeros((BANDS, n_batch, 16, 2 * LEVEL, F), np.float16)
    for l in range(LEVEL):
        for ch, v in ((l, fas[l]), (LEVEL + l, fbs[l])):
            sf = np.zeros(BANDS * c_band, np.float16)
            sf[slot_global] = v
            frcm[:, :, :, ch, :] = _to_lerp_layout(
                sf.reshape(BANDS, c_band), n_batch)

    # bands -> (rank, pass, core): band = 32r + 8p + k
    tab_r = tab.reshape(N_RANKS, N_PASSES, N_Q7, 16, ETOT)
    idx_r = (idxm.reshape(N_RANKS, N_PASSES, N_Q7, n_batch, 16, F)
             .transpose(0, 1, 3, 2, 4, 5)
             .reshape(N_RANKS, N_PASSES, n_batch, 128, F))
    frc_r = (frcm.reshape(N_RANKS, N_PASSES, N_Q7, n_batch, 16, 2 * LEVEL, F)
             .transpose(0, 1, 3, 2, 4, 5, 6)
             .reshape(N_RANKS, N_PASSES, n_batch, 128, 2 * LEVEL, F))

    in_maps = [
        {"tab": np.ascontiguousarray(tab_r[r]),
         "idx": np.ascontiguousarray(idx_r[r]),
         "frc": np.ascontiguousarray(frc_r[r])}
        for r in range(N_RANKS)
    ]
    kres = run_bass_kernel_spmd(nc, in_maps, list(range(N_RANKS)))
    LAST_RESULT = kres
    results = kres.results
    res = np.stack([results[r]["out"] for r in range(N_RANKS)])
    # [R, P, nb, 128(k,q), L, F] -> [BANDS, c_band(bi,j,q), LEVEL]
    res = (res.reshape(N_RANKS, N_PASSES, n_batch, N_Q7, 16, LEVEL, F)
           .transpose(0, 1, 3, 2, 6, 4, 5)
           .reshape(BANDS * c_band, LEVEL))

    out = res[slot_global].astype(np.float32) * \
        (np.asarray(deq, np.float32)[None, :])
    assert out.shape == (n, LEVEL)
    return out


# revision 29
# speedup vs baseline: 1.1452x; 1.0143x over previous
"""Trainium2 Bass kernel for multi-level bilinear grid interpolation
(embedding_lookup, nn_COOLCHIC_INTERP_ENC).

Strategy (v3):
  - 8 NeuronCores, data-parallel over query points, sharded spatially by
    latitude into 256 count-balanced bands (8 ranks x 4 passes x 8 gpsimd
    cores; equal-angle fallback for pathological inputs).
  - KEY TRICK: floor(t/res_l) == floor(t/res_0) >> l exactly (res_l are
    powers of two and t/res_l is an exact f32 scaling), so ONE level-0
    cell index (row-in-band, col) identifies every level's bilinear quad.
    ap_gather reads each partition's own table row, so partition 16k+q of
    gpsimd core k holds a table for level q%8 whose entry e is that
    level's 2x2 quad for L0-cell e -> a single d=1 f32 ap_gather per
    batch fetches ALL 8 levels' quads for the core's 16*F points.
  - Quads are 4 x int8 (per-level symmetric quantization, error
    <= absmax/254 ~ 0.4%) packed in one f32 word.
  - Gather indices (int16) and per-level lerp fractions (fp16) are
    host-precomputed directly in engine layouts; the per-level gather
    output de-interleave is ONE SBUF->SBUF DMA with 2KB-contiguous
    descriptors (partition 16k+l holds level l's value for the whole
    core stream; stream slot q*F+j belongs to lerp partition 16k+q).
  - DVE does the 9-op bilinear lerp per level (int8 corners, f32
    intermediates, fp16 fracs/result); host de-quantizes.
"""

import sys

sys.path.insert(0, "/opt/trn_rl_repo")

import numpy as np

from concourse import bacc, bass, mybir
import concourse.tile as tile

# ---------------------------------------------------------------- constants
H_GRID, W_GRID, LEVEL, RES = 721, 1440, 8, 0.25
N_RANKS = 8
N_PASSES = 4
N_Q7 = 8
BANDS = N_RANKS * N_PASSES * N_Q7  # 256
F = 496                   # points per partition per batch
NI = 16 * F               # gather stream length per core (= points/core/batch)
CAP0 = 4                  # level-0 rows per band (max floor-span, exact)
ETOT = CAP0 * W_GRID      # table entries per partition (L0 cells)
NMC = 1 + 2 * LEVEL       # meta channels: idx + (fa, fb) per level

F32 = mybir.dt.float32
F16 = mybir.dt.float16
I16 = mybir.dt.int16
I8 = mybir.dt.int8


def _res(l):
    return RES * (2.0 ** l)


# ---------------------------------------------------------------- device kernel
def build_kernel(n_batch):
    """Per-rank SPMD Bass program. c_band = n_batch * NI points per band."""
    nc = bacc.Bacc(None, target_bir_lowering=False)

    # tables come 16x-per-level-replicated from the host: [8(k), 16(q), ETOT]
    # with partition 16k+q holding band k's level-(q%8) quad table.
    tab_t = nc.declare_dram_parameter(
        "tab", [N_PASSES, N_Q7, 16, ETOT], F32, False)
    idx_t = nc.declare_dram_parameter(
        "idx", [N_PASSES, n_batch, 128, F], I16, False)
    frc_t = nc.declare_dram_parameter(
        "frc", [N_PASSES, n_batch, 128, 2 * LEVEL, F], F16, False)
    out_t = nc.declare_dram_parameter(
        "out", [N_PASSES, n_batch, 128, LEVEL, F], F16, True)

    sub = mybir.AluOpType.subtract
    add = mybir.AluOpType.add
    mult = mybir.AluOpType.mult

    from contextlib import ExitStack

    nbat = N_PASSES * n_batch           # global batch index g = p*n_batch+bi
    LOOK = 2                            # deint/unpack emitted LOOK items early

    with tile.TileContext(nc) as tc, ExitStack() as es:
        ptab = es.enter_context(tc.tile_pool(name="ptab", bufs=2))
        pdst = es.enter_context(tc.tile_pool(name="pdst", bufs=2))
        pm = es.enter_context(tc.tile_pool(name="pm", bufs=2))
        pq = es.enter_context(tc.tile_pool(name="pq", bufs=3))
        pr = es.enter_context(tc.tile_pool(name="pr", bufs=2))
        pt = es.enter_context(tc.tile_pool(name="pt", bufs=3))

        # per-global-batch state created lazily in emission order
        tabs_of, dst_of, fr_of, res_of, crn_of = {}, {}, {}, {}, {}

        def emit_batch_front(g):
            """prefetch idx/frc, (new pass: table), and the gather for g."""
            p, bi = divmod(g, n_batch)
            if bi == 0:
                # NOTE: all 16 partition rows must hold valid tables — the
                # gather ucode mixes the two replicated level copies (q and
                # q+8), so garbage there corrupts consumed outputs.
                tabs = ptab.tile([128, ETOT], F32, tag="tabs")
                for c in range(4):
                    nc.sync.dma_start(out=tabs[32 * c:32 * (c + 1)],
                                      in_=tab_t[p, 2 * c:2 * c + 2])
                tabs_of[p] = tabs
            ix = pm.tile([128, F], I16, tag="idx")
            nc.sync.dma_start(out=ix[:], in_=idx_t[p, bi])
            fr = pm.tile([128, 2 * LEVEL, F], F16, tag="frc")
            nc.sync.dma_start(out=fr[:], in_=frc_t[p, bi])
            fr_of[g] = fr
            dst = pdst.tile([128, NI], F32, tag="dst")
            nc.gpsimd.ap_gather(
                dst[:].rearrange("p (n d) -> p n d", d=1),
                tabs_of[p][:].rearrange("p (n d) -> p n d", d=1),
                ix[:],
                channels=128, num_elems=ETOT, d=1, num_idxs=NI)
            dst_of[g] = dst

        def emit_fetch(g, l):
            """de-interleave levels l, l+1 of batch g + int8->fp16 corner
            unpack, all on Activation so their gather-wait can't block SP."""
            quad = pq.tile([128, 2, F], F32, tag="quad")
            nc.scalar.dma_start(out=quad[:, 0, :], in_=dst_of[g][l::16])
            nc.scalar.dma_start(out=quad[:, 1, :], in_=dst_of[g][l + 1::16])
            crn = pq.tile([128, 4, 2 * F], F16, tag="crn")
            nc.scalar.copy(out=crn[:].rearrange("p r j -> p j r"),
                           in_=quad[:].rearrange("p v j -> p (v j)").bitcast(I8))
            crn_of[(g, l)] = crn

        def emit_lerp(g, l):
            p, bi = divmod(g, n_batch)
            crn = crn_of.pop((g, l))
            v00, v10, v01, v11 = (crn[:, c, :] for c in range(4))
            fr = fr_of[g]
            fa = fr[:, l:l + 2, :].rearrange("p v j -> p (v j)")
            fb = fr[:, LEVEL + l:LEVEL + l + 2, :].rearrange(
                "p v j -> p (v j)")
            t1 = pt.tile([128, 2 * F], F16, tag="t1")
            t2 = pt.tile([128, 2 * F], F16, tag="t2")
            res = pr.tile([128, 2, F], F16, tag="res")
            V = nc.vector
            # v_f = v00 + fb*(v01 - v00); v_c = v10 + fb*(v11 - v10)
            V.tensor_tensor(out=t1[:], in0=v01, in1=v00, op=sub)
            V.tensor_tensor(out=t1[:], in0=t1[:], in1=fb, op=mult)
            V.tensor_tensor(out=t1[:], in0=t1[:], in1=v00, op=add)
            V.tensor_tensor(out=t2[:], in0=v11, in1=v10, op=sub)
            V.tensor_tensor(out=t2[:], in0=t2[:], in1=fb, op=mult)
            V.tensor_tensor(out=t2[:], in0=t2[:], in1=v10, op=add)
            # out = v_f + fa*(v_c - v_f)
            V.tensor_tensor(out=t2[:], in0=t2[:], in1=t1[:], op=sub)
            V.tensor_tensor(out=t2[:], in0=t2[:], in1=fa, op=mult)
            V.tensor_tensor(out=res[:].rearrange("p v j -> p (v j)"),
                            in0=t2[:], in1=t1[:], op=add)
            nc.sync.dma_start(out=out_t[p, bi, :, l:l + 2, :], in_=res[:])
            if l == LEVEL - 2:
                fr_of.pop(g)

        items = [(g, l) for g in range(nbat) for l in range(0, LEVEL, 2)]
        emit_batch_front(0)
        for k in range(len(items) + LOOK):
            if k < len(items):
                g, l = items[k]
                # keep the NEXT batch's gather a full batch ahead
                if l == 0 and g + 1 < nbat:
                    emit_batch_front(g + 1)
                emit_fetch(g, l)
            if k >= LOOK:
                emit_lerp(*items[k - LOOK])

    nc.compile()
    return nc


# ---------------------------------------------------------------- host tables
def quantize(emb):
    """emb [LEVEL,H,W] f32 -> int8 grids + per-level dequant factors."""
    scl = np.abs(emb).max(axis=(1, 2))
    scl = np.where(scl > 0, scl, 1.0).astype(np.float64)
    q8 = np.clip(np.rint(emb * (127.0 / scl)[:, None, None]),
                 -127, 127).astype(np.int8)
    return q8, (scl / 127.0).astype(np.float64)


def equal_angle_boundaries():
    """Band boundaries in t = 90 - lat space; exact f32 values."""
    return np.float32(np.arange(1, BANDS) * 45.0 / 64.0)


def quantile_boundaries(t32):
    """Count-balanced boundaries: the sorted t32 at the 256-quantile cuts."""
    ts = np.sort(t32)
    return ts[(np.arange(1, BANDS, dtype=np.int64) * ts.size) // BANDS]


def band_row_starts(bnd):
    """RS0[b] = floor(lo_b / RES), exact: lo_b is an exact f32 and 1/RES a
    power of two, so the f64 product is exact."""
    lo = np.concatenate([[np.float32(0.0)], bnd]).astype(np.float64)
    return np.floor(lo / RES).astype(np.int64)


def build_tables(q8, RS0):
    """-> tab [BANDS, 16, ETOT] f32; partition q holds level q%8's quad
    table over L0 cells: entry (r0loc, w0) = level-l quad at
    (h_l, w_l) = ((RS0+r0loc)>>l, w0>>l), int8x4-packed."""
    tab = np.zeros((BANDS, N_Q7, ETOT, 4), np.int8)  # [band, level, e, 4]
    w0 = np.arange(W_GRID)
    for l in range(LEVEL):
        rows0 = RS0[:, None] + np.arange(CAP0)[None, :]       # [BANDS, CAP0]
        hl = np.clip(rows0 >> l, 0, H_GRID - 1)
        hl1 = np.clip((rows0 >> l) + 1, 0, H_GRID - 1)
        wl = w0 >> l
        wl1 = np.minimum(wl + 1, W_GRID - 1)
        g0 = q8[l][hl]                                        # [BANDS,CAP0,W]
        g1 = q8[l][hl1]
        ent = np.stack([g0[:, :, wl], g1[:, :, wl], g0[:, :, wl1],
                        g1[:, :, wl1]], axis=-1)              # [B,CAP0,W,4]
        tab[:, l] = ent.reshape(BANDS, ETOT, 4)
    # replicate levels onto partitions 8..15, view as f32 words
    tab16 = np.concatenate([tab, tab], axis=1)                # [B, 16, E, 4]
    return np.ascontiguousarray(tab16).view('<f4').reshape(BANDS, 16, ETOT)


# ---------------------------------------------------------------- host points
def point_data(t32, lon, band, RS0):
    """idx int16 [N] (L0 cell id in band window) + per-level fracs fp16,
    plus in-window validity. Mirrors the f32 reference exactly."""
    a0 = t32 / np.float32(RES)
    fl0 = np.floor(a0)
    raw = np.clip(fl0, 0, H_GRID - 1).astype(np.int64) - RS0[band]
    ok = bool(raw.size == 0 or (raw.min() >= 0 and raw.max() <= CAP0 - 1))
    row_local = np.clip(raw, 0, CAP0 - 1)
    o0 = lon / np.float32(RES)
    w0 = np.clip(np.floor(o0), 0, W_GRID - 1).astype(np.int64)
    idx = (row_local * W_GRID + w0).astype(np.int16)
    fas, fbs = [], []
    for l in range(LEVEL):
        r = np.float32(_res(l))
        a = t32 / r
        lat_f = np.clip(np.floor(a), 0, H_GRID - 1)
        fas.append((a - lat_f).astype(np.float16))
        o = lon / r
        wf = np.clip(np.floor(o), 0, W_GRID - 1)
        fbs.append((o - wf).astype(np.float16))
    return idx, fas, fbs, ok


def slot_assign(band, c_band):
    """slot_global [N]: slot index in [0, BANDS*c_band) per point."""
    order = np.argsort(band, kind="stable")
    counts = np.bincount(band, minlength=BANDS)
    starts = np.zeros(BANDS, np.int64)
    starts[1:] = np.cumsum(counts)[:-1]
    pos_sorted = np.arange(band.size, dtype=np.int64) - starts[band[order]]
    slot_global = np.empty(band.size, np.int64)
    slot_global[order] = band[order] * c_band + pos_sorted
    return slot_global, counts


def _to_lerp_layout(slots, n_batch):
    """[BANDS, c_band] -> [BANDS, nb, 16(q), F(j)]; slot s=(bi*F+j)*16+q."""
    return (slots.reshape(BANDS, n_batch, F, 16).transpose(0, 1, 3, 2))


def _to_idx_layout(slots, n_batch):
    """[BANDS, c_band] -> [BANDS, nb, 16(m), F(c)]; stream i = q*F+j,
    written at partition m=i%16, col c=i//16."""
    lerp = _to_lerp_layout(slots, n_batch)          # [B, nb, q, j]
    stream = lerp.reshape(BANDS, n_batch, NI)       # i = q*F + j
    return stream.reshape(BANDS, n_batch, F, 16).transpose(0, 1, 3, 2)


# ---------------------------------------------------------------- entry point
_NC_CACHE = {}
LAST_RESULT = None


def kernel(x, embeddings):
    global LAST_RESULT
    from concourse.bass_utils import run_bass_kernel_spmd

    x = np.ascontiguousarray(np.asarray(x), dtype=np.float32)
    emb = np.asarray(embeddings, dtype=np.float32)
    n = x.shape[0]

    q8, deq = quantize(emb)
    lat = x[:, 0].astype(np.float32)
    lon = x[:, 1].astype(np.float32)
    t32 = np.float32(90.0) - lat

    # count-balanced bands minimize padded-slot waste; fall back to
    # equal-angle bands if any L0 floor escapes its band's 4-row window
    # (only possible for pathological latitude distributions).
    for bnd in (quantile_boundaries(t32), equal_angle_boundaries()):
        band = np.searchsorted(bnd, t32, side="right").astype(np.int64)
        RS0 = band_row_starts(bnd)
        idx, fas, fbs, ok = point_data(t32, lon, band, RS0)
        if ok:
            break
    tab = build_tables(q8, RS0)                     # [BANDS, 16, ETOT] f32

    counts = np.bincount(band, minlength=BANDS)
    n_batch = 1
    while n_batch * NI < counts.max():
        n_batch += 1
    c_band = n_batch * NI

    if n_batch not in _NC_CACHE:
        _NC_CACHE[n_batch] = build_kernel(n_batch)
    nc = _NC_CACHE[n_batch]

    slot_global, counts = slot_assign(band, c_band)

    idxm = np.zeros((BANDS, n_batch, 16, F), np.int16)
    sl = np.zeros(BANDS * c_band, np.int16)
    sl[slot_global] = idx
    idxm[:] = _to_idx_layout(sl.reshape(BANDS, c_band), n_batch)
    frcm = np.zeros((BANDS, n_batch, 16, 2 * LEVEL, F), np.float16)
    for l in range(LEVEL):
        for ch, v in ((l, fas[l]), (LEVEL + l, fbs[l])):
            sf = np.zeros(BANDS * c_band, np.float16)
            sf[slot_global] = v
            frcm[:, :, :, ch, :] = _to_lerp_layout(
                sf.reshape(BANDS, c_band), n_batch)

    # bands -> (rank, pass, core): band = 32r + 8p + k
    tab_r = tab.reshape(N_RANKS, N_PASSES, N_Q7, 16, ETOT)
    idx_r = (idxm.reshape(N_RANKS, N_PASSES, N_Q7, n_batch, 16, F)
             .transpose(0, 1, 3, 2, 4, 5)
             .reshape(N_RANKS, N_PASSES, n_batch, 128, F))
    frc_r = (frcm.reshape(N_RANKS, N_PASSES, N_Q7, n_batch, 16, 2 * LEVEL, F)
             .transpose(0, 1, 3, 2, 4, 5, 6)
             .reshape(N_RANKS, N_PASSES, n_batch, 128, 2 * LEVEL, F))

    in_maps = [
        {"tab": np.ascontiguousarray(tab_r[r]),
         "idx": np.ascontiguousarray(idx_r[r]),
         "frc": np.ascontiguousarray(frc_r[r])}
        for r in range(N_RANKS)
    ]
    kres = run_bass_kernel_spmd(nc, in_maps, list(range(N_RANKS)))
    LAST_RESULT = kres
    results = kres.results
    res = np.stack([results[r]["out"] for r in range(N_RANKS)])
    # [R, P, nb, 128(k,q), L, F] -> [BANDS, c_band(bi,j,q), LEVEL]
    res = (res.reshape(N_RANKS, N_PASSES, n_batch, N_Q7, 16, LEVEL, F)
           .transpose(0, 1, 3, 2, 6, 4, 5)
           .reshape(BANDS * c_band, LEVEL))

    out = res[slot_global].astype(np.float32) * \
        (np.asarray(deq, np.float32)[None, :])
    assert out.shape == (n, LEVEL)
    return out


# revision 32
# speedup vs baseline: 1.7648x; 1.5411x over previous
"""Trainium2 Bass kernel for multi-level bilinear grid interpolation
(embedding_lookup, nn_COOLCHIC_INTERP_ENC).

Strategy (v3):
  - 8 NeuronCores, data-parallel over query points, sharded spatially by
    latitude into 256 count-balanced bands (8 ranks x 4 passes x 8 gpsimd
    cores; equal-angle fallback for pathological inputs).
  - KEY TRICK: floor(t/res_l) == floor(t/res_0) >> l exactly (res_l are
    powers of two and t/res_l is an exact f32 scaling), so ONE level-0
    cell index (row-in-band, col) identifies every level's bilinear quad.
    ap_gather reads each partition's own table row, so partition 16k+q of
    gpsimd core k holds a table for level q%8 whose entry e is that
    level's 2x2 quad for L0-cell e -> a single d=1 f32 ap_gather per
    batch fetches ALL 8 levels' quads for the core's 16*F points.
  - Quads are 4 x int8 (per-level symmetric quantization, error
    <= absmax/254 ~ 0.4%) packed in one f32 word.
  - Gather indices (int16) and per-level lerp fractions (fp16) are
    host-precomputed directly in engine layouts; the per-level gather
    output de-interleave is ONE SBUF->SBUF DMA with 2KB-contiguous
    descriptors (partition 16k+l holds level l's value for the whole
    core stream; stream slot q*F+j belongs to lerp partition 16k+q).
  - DVE does the 9-op bilinear lerp per level (int8 corners, f32
    intermediates, fp16 fracs/result); host de-quantizes.
"""

import sys

sys.path.insert(0, "/opt/trn_rl_repo")

import numpy as np

from concourse import bacc, bass, mybir
import concourse.tile as tile

# ---------------------------------------------------------------- constants
H_GRID, W_GRID, LEVEL, RES = 721, 1440, 8, 0.25
N_RANKS = 8
N_PASSES = 4
N_Q7 = 8
BANDS = N_RANKS * N_PASSES * N_Q7  # 256
F = 496                   # points per partition per batch
NI = 16 * F               # gather stream length per core (= points/core/batch)
CAP0 = 4                  # level-0 rows per band (max floor-span, exact)
ETOT = CAP0 * W_GRID      # table entries per partition (L0 cells)
NMC = 1 + 2 * LEVEL       # meta channels: idx + (fa, fb) per level

F32 = mybir.dt.float32
F16 = mybir.dt.float16
I16 = mybir.dt.int16
I8 = mybir.dt.int8


def _res(l):
    return RES * (2.0 ** l)


# ---------------------------------------------------------------- device kernel
def build_kernel(n_batch):
    """Per-rank SPMD Bass program. c_band = n_batch * NI points per band."""
    nc = bacc.Bacc(None, target_bir_lowering=False)

    # tables come 16x-per-level-replicated from the host: [8(k), 16(q), ETOT]
    # with partition 16k+q holding band k's level-(q%8) quad table.
    tab_t = nc.declare_dram_parameter(
        "tab", [N_PASSES, N_Q7, 16, ETOT], F32, False)
    idx_t = nc.declare_dram_parameter(
        "idx", [N_PASSES, n_batch, 128, F], I16, False)
    frc_t = nc.declare_dram_parameter(
        "frc", [N_PASSES, n_batch, 128, 2 * LEVEL, F], F16, False)
    out_t = nc.declare_dram_parameter(
        "out", [N_PASSES, n_batch, 128, LEVEL, F], F16, True)

    sub = mybir.AluOpType.subtract
    add = mybir.AluOpType.add
    mult = mybir.AluOpType.mult

    from contextlib import ExitStack

    nbat = N_PASSES * n_batch           # global batch index g = p*n_batch+bi
    LOOK = 2                            # deint/unpack emitted LOOK items early

    with tile.TileContext(nc) as tc, ExitStack() as es:
        ptab = es.enter_context(tc.tile_pool(name="ptab", bufs=2))
        pdst = es.enter_context(tc.tile_pool(name="pdst", bufs=2))
        pm = es.enter_context(tc.tile_pool(name="pm", bufs=2))
        pq = es.enter_context(tc.tile_pool(name="pq", bufs=3))
        pr = es.enter_context(tc.tile_pool(name="pr", bufs=2))
        pt = es.enter_context(tc.tile_pool(name="pt", bufs=3))

        # per-global-batch state created lazily in emission order
        tabs_of, dst_of, fr_of, res_of, crn_of = {}, {}, {}, {}, {}

        def emit_batch_front(g):
            """prefetch idx/frc, (new pass: table), and the gather for g."""
            p, bi = divmod(g, n_batch)
            if bi == 0:
                # NOTE: all 16 partition rows must hold valid tables — the
                # gather ucode mixes the two replicated level copies (q and
                # q+8), so garbage there corrupts consumed outputs.
                tabs = ptab.tile([128, ETOT], F32, tag="tabs")
                for c in range(4):
                    nc.sync.dma_start(out=tabs[32 * c:32 * (c + 1)],
                                      in_=tab_t[p, 2 * c:2 * c + 2])
                tabs_of[p] = tabs
            ix = pm.tile([128, F], I16, tag="idx")
            nc.sync.dma_start(out=ix[:], in_=idx_t[p, bi])
            fr = pm.tile([128, 2 * LEVEL, F], F16, tag="frc")
            nc.sync.dma_start(out=fr[:], in_=frc_t[p, bi])
            fr_of[g] = fr
            dst = pdst.tile([128, NI], F32, tag="dst")
            nc.gpsimd.ap_gather(
                dst[:].rearrange("p (n d) -> p n d", d=1),
                tabs_of[p][:].rearrange("p (n d) -> p n d", d=1),
                ix[:],
                channels=128, num_elems=ETOT, d=1, num_idxs=NI)
            dst_of[g] = dst

        def emit_fetch(g, l, nl):
            """de-interleave levels l..l+nl-1 of batch g + int8->fp16 corner
            unpack, all on Activation so their gather-wait can't block SP."""
            quad = pq.tile([128, 2, F], F32, tag="quad")
            for v in range(nl):
                nc.scalar.dma_start(out=quad[:, v, :], in_=dst_of[g][l + v::16])
            crn = pq.tile([128, 4, 2 * F], F16, tag="crn")
            nc.scalar.copy(
                out=crn[:, :, :nl * F].rearrange("p r j -> p j r"),
                in_=quad[:, :nl, :].rearrange("p v j -> p (v j)").bitcast(I8))
            crn_of[(g, l)] = crn

        def emit_lerp(g, l, nl):
            p, bi = divmod(g, n_batch)
            crn = crn_of.pop((g, l))
            v00, v10, v01, v11 = (crn[:, c, :nl * F] for c in range(4))
            fr = fr_of[g]
            fa = fr[:, l:l + nl, :].rearrange("p v j -> p (v j)")
            fb = fr[:, LEVEL + l:LEVEL + l + nl, :].rearrange(
                "p v j -> p (v j)")
            t1f = pt.tile([128, 2 * F], F16, tag="t1")
            t2f = pt.tile([128, 2 * F], F16, tag="t2")
            t1 = t1f[:, :nl * F]
            t2 = t2f[:, :nl * F]
            res = pr.tile([128, 2, F], F16, tag="res")
            V = nc.vector
            # v_f = v00 + fb*(v01 - v00); v_c = v10 + fb*(v11 - v10)
            V.tensor_tensor(out=t1, in0=v01, in1=v00, op=sub)
            V.tensor_tensor(out=t1, in0=t1, in1=fb, op=mult)
            V.tensor_tensor(out=t1, in0=t1, in1=v00, op=add)
            V.tensor_tensor(out=t2, in0=v11, in1=v10, op=sub)
            V.tensor_tensor(out=t2, in0=t2, in1=fb, op=mult)
            V.tensor_tensor(out=t2, in0=t2, in1=v10, op=add)
            # out = v_f + fa*(v_c - v_f)
            V.tensor_tensor(out=t2, in0=t2, in1=t1, op=sub)
            V.tensor_tensor(out=t2, in0=t2, in1=fa, op=mult)
            V.tensor_tensor(
                out=res[:, :nl, :].rearrange("p v j -> p (v j)"),
                in0=t2, in1=t1, op=add)
            nc.sync.dma_start(out=out_t[p, bi, :, l:l + nl, :],
                              in_=res[:, :nl, :])
            if l + nl == LEVEL:
                fr_of.pop(g)

        # level-pair items, except the very first and last batches start/end
        # with single levels to shorten the pipeline fill and drain chains.
        items = []
        for g in range(nbat):
            if g == 0:
                items += [(g, 0, 1), (g, 1, 1)]
                items += [(g, l, 2) for l in range(2, LEVEL, 2)]
            elif g == nbat - 1:
                items += [(g, l, 2) for l in range(0, LEVEL - 2, 2)]
                items += [(g, LEVEL - 2, 1), (g, LEVEL - 1, 1)]
            else:
                items += [(g, l, 2) for l in range(0, LEVEL, 2)]
        emit_batch_front(0)
        for k in range(len(items) + LOOK):
            if k < len(items):
                g, l, nl = items[k]
                # keep the NEXT batch's gather a full batch ahead
                if l == 0 and g + 1 < nbat:
                    emit_batch_front(g + 1)
                emit_fetch(g, l, nl)
            if k >= LOOK:
                emit_lerp(*items[k - LOOK])

    nc.compile()
    return nc


# ---------------------------------------------------------------- host tables
def quantize(emb):
    """emb [LEVEL,H,W] f32 -> int8 grids + per-level dequant factors."""
    scl = np.abs(emb).max(axis=(1, 2))
    scl = np.where(scl > 0, scl, 1.0).astype(np.float64)
    q8 = np.clip(np.rint(emb * (127.0 / scl)[:, None, None]),
                 -127, 127).astype(np.int8)
    return q8, (scl / 127.0).astype(np.float64)


def equal_angle_boundaries():
    """Band boundaries in t = 90 - lat space; exact f32 values."""
    return np.float32(np.arange(1, BANDS) * 45.0 / 64.0)


def quantile_boundaries(t32):
    """Count-balanced boundaries: the sorted t32 at the 256-quantile cuts."""
    ts = np.sort(t32)
    return ts[(np.arange(1, BANDS, dtype=np.int64) * ts.size) // BANDS]


def band_row_starts(bnd):
    """RS0[b] = floor(lo_b / RES), exact: lo_b is an exact f32 and 1/RES a
    power of two, so the f64 product is exact."""
    lo = np.concatenate([[np.float32(0.0)], bnd]).astype(np.float64)
    return np.floor(lo / RES).astype(np.int64)


def build_tables(q8, RS0):
    """-> tab [BANDS, 16, ETOT] f32; partition q holds level q%8's quad
    table over L0 cells: entry (r0loc, w0) = level-l quad at
    (h_l, w_l) = ((RS0+r0loc)>>l, w0>>l), int8x4-packed."""
    tab = np.zeros((BANDS, N_Q7, ETOT, 4), np.int8)  # [band, level, e, 4]
    w0 = np.arange(W_GRID)
    for l in range(LEVEL):
        rows0 = RS0[:, None] + np.arange(CAP0)[None, :]       # [BANDS, CAP0]
        hl = np.clip(rows0 >> l, 0, H_GRID - 1)
        hl1 = np.clip((rows0 >> l) + 1, 0, H_GRID - 1)
        wl = w0 >> l
        wl1 = np.minimum(wl + 1, W_GRID - 1)
        g0 = q8[l][hl]                                        # [BANDS,CAP0,W]
        g1 = q8[l][hl1]
        ent = np.stack([g0[:, :, wl], g1[:, :, wl], g0[:, :, wl1],
                        g1[:, :, wl1]], axis=-1)              # [B,CAP0,W,4]
        tab[:, l] = ent.reshape(BANDS, ETOT, 4)
    # replicate levels onto partitions 8..15, view as f32 words
    tab16 = np.concatenate([tab, tab], axis=1)                # [B, 16, E, 4]
    return np.ascontiguousarray(tab16).view('<f4').reshape(BANDS, 16, ETOT)


# ---------------------------------------------------------------- host points
def point_data(t32, lon, band, RS0):
    """idx int16 [N] (L0 cell id in band window) + per-level fracs fp16,
    plus in-window validity. Mirrors the f32 reference exactly."""
    a0 = t32 / np.float32(RES)
    fl0 = np.floor(a0)
    raw = np.clip(fl0, 0, H_GRID - 1).astype(np.int64) - RS0[band]
    ok = bool(raw.size == 0 or (raw.min() >= 0 and raw.max() <= CAP0 - 1))
    row_local = np.clip(raw, 0, CAP0 - 1)
    o0 = lon / np.float32(RES)
    w0 = np.clip(np.floor(o0), 0, W_GRID - 1).astype(np.int64)
    idx = (row_local * W_GRID + w0).astype(np.int16)
    fas, fbs = [], []
    for l in range(LEVEL):
        r = np.float32(_res(l))
        a = t32 / r
        lat_f = np.clip(np.floor(a), 0, H_GRID - 1)
        fas.append((a - lat_f).astype(np.float16))
        o = lon / r
        wf = np.clip(np.floor(o), 0, W_GRID - 1)
        fbs.append((o - wf).astype(np.float16))
    return idx, fas, fbs, ok


def slot_assign(band, c_band):
    """slot_global [N]: slot index in [0, BANDS*c_band) per point."""
    order = np.argsort(band, kind="stable")
    counts = np.bincount(band, minlength=BANDS)
    starts = np.zeros(BANDS, np.int64)
    starts[1:] = np.cumsum(counts)[:-1]
    pos_sorted = np.arange(band.size, dtype=np.int64) - starts[band[order]]
    slot_global = np.empty(band.size, np.int64)
    slot_global[order] = band[order] * c_band + pos_sorted
    return slot_global, counts


def _to_lerp_layout(slots, n_batch):
    """[BANDS, c_band] -> [BANDS, nb, 16(q), F(j)]; slot s=(bi*F+j)*16+q."""
    return (slots.reshape(BANDS, n_batch, F, 16).transpose(0, 1, 3, 2))


def _to_idx_layout(slots, n_batch):
    """[BANDS, c_band] -> [BANDS, nb, 16(m), F(c)]; stream i = q*F+j,
    written at partition m=i%16, col c=i//16."""
    lerp = _to_lerp_layout(slots, n_batch)          # [B, nb, q, j]
    stream = lerp.reshape(BANDS, n_batch, NI)       # i = q*F + j
    return stream.reshape(BANDS, n_batch, F, 16).transpose(0, 1, 3, 2)


# ---------------------------------------------------------------- entry point
_NC_CACHE = {}
LAST_RESULT = None


def kernel(x, embeddings):
    global LAST_RESULT
    from concourse.bass_utils import run_bass_kernel_spmd

    x = np.ascontiguousarray(np.asarray(x), dtype=np.float32)
    emb = np.asarray(embeddings, dtype=np.float32)
    n = x.shape[0]

    q8, deq = quantize(emb)
    lat = x[:, 0].astype(np.float32)
    lon = x[:, 1].astype(np.float32)
    t32 = np.float32(90.0) - lat

    # count-balanced bands minimize padded-slot waste; fall back to
    # equal-angle bands if any L0 floor escapes its band's 4-row window
    # (only possible for pathological latitude distributions).
    for bnd in (quantile_boundaries(t32), equal_angle_boundaries()):
        band = np.searchsorted(bnd, t32, side="right").astype(np.int64)
        RS0 = band_row_starts(bnd)
        idx, fas, fbs, ok = point_data(t32, lon, band, RS0)
        if ok:
            break
    tab = build_tables(q8, RS0)                     # [BANDS, 16, ETOT] f32

    counts = np.bincount(band, minlength=BANDS)
    n_batch = 1
    while n_batch * NI < counts.max():
        n_batch += 1
    c_band = n_batch * NI

    if n_batch not in _NC_CACHE:
        _NC_CACHE[n_batch] = build_kernel(n_batch)
    nc = _NC_CACHE[n_batch]

    slot_global, counts = slot_assign(band, c_band)

    idxm = np.zeros((BANDS, n_batch, 16, F), np.int16)
    sl = np.zeros(BANDS * c_band, np.int16)
    sl[slot_global] = idx
    idxm[:] = _to_idx_layout(sl.reshape(BANDS, c_band), n_batch)
    frcm = np.zeros((BANDS, n_batch, 16, 2 * LEVEL, F), np.float16)
    for l in range(LEVEL):
        for ch, v in ((l, fas[l]), (LEVEL + l, fbs[l])):
            sf = np.zeros(BANDS * c_band, np.float16)
            sf[slot_global] = v
            frcm[:, :, :, ch, :] = _to_lerp_layout(
                sf.reshape(BANDS, c_band), n_batch)

    # bands -> (rank, pass, core): band = 32r + 8p + k
    tab_r = tab.reshape(N_RANKS, N_PASSES, N_Q7, 16, ETOT)
    idx_r = (idxm.reshape(N_RANKS, N_PASSES, N_Q7, n_batch, 16, F)
             .transpose(0, 1, 3, 2, 4, 5)
             .reshape(N_RANKS, N_PASSES, n_batch, 128, F))
    frc_r = (frcm.reshape(N_RANKS, N_PASSES, N_Q7, n_batch, 16, 2 * LEVEL, F)
             .transpose(0, 1, 3, 2, 4, 5, 6)
             .reshape(N_RANKS, N_PASSES, n_batch, 128, 2 * LEVEL, F))

    in_maps = [
        {"tab": np.ascontiguousarray(tab_r[r]),
         "idx": np.ascontiguousarray(idx_r[r]),
         "frc": np.ascontiguousarray(frc_r[r])}
        for r in range(N_RANKS)
    ]
    kres = run_bass_kernel_spmd(nc, in_maps, list(range(N_RANKS)))
    LAST_RESULT = kres
    results = kres.results
    res = np.stack([results[r]["out"] for r in range(N_RANKS)])
    # [R, P, nb, 128(k,q), L, F] -> [BANDS, c_band(bi,j,q), LEVEL]
    res = (res.reshape(N_RANKS, N_PASSES, n_batch, N_Q7, 16, LEVEL, F)
           .transpose(0, 1, 3, 2, 6, 4, 5)
           .reshape(BANDS * c_band, LEVEL))

    out = res[slot_global].astype(np.float32) * \
        (np.asarray(deq, np.float32)[None, :])
    assert out.shape == (n, LEVEL)
    return out
